# revision 7
# baseline (speedup 1.0000x reference)
"""Trainium2 Bass kernel v7: 2D parallel-beam forward projection (Radon transform).

Input:  x [2, 256, 256, 1] float32
Output: sinogram [2, 180, 363, 1] float32

Strategy (8 NeuronCores, SPMD):
  - Angles interleaved across cores; detectors interleaved across the 8
    GPSIMD stream-groups (group g: d % 8 == g), stream g on partitions
    16g..16g+15 (8 window rows x 2 batches).
  - Per-angle transpose trick: for |sin| > |cos| the projection is computed
    bit-exactly on the transposed image with (c,s) swapped, so the ray's
    y-step is always >= 0.707.
  - Wide cells: a cell is (8-row window u, 7-slot x-window xq). One gather
    index fetches d=4 f32 = 8 fp16 pixels img[y, 7*xq .. 7*xq+7] for all 16
    partition rows; any sample with x0 in [7*xq, 7*xq+6] is covered. Cells
    merge all samples of a ray touching the same (u, xq): ~2.9x fewer gather
    indices than per-(u,x) cells. ap_gather cost is per-index command-bound,
    so wider cells are nearly free.
  - Per-(cell,row) weights: 8 fp16 slot weights (folded y-lerp x x-lerp).
  - Device: ap_gather -> one DVE mul (f16*f16 -> f32 exact) -> one segmented
    f32 reduce per chunk -> TensorE folds 16 partition rows into
    per-(stream,batch) ray sums.
"""
import os
import sys
from contextlib import ExitStack

import numpy as np

for p in ("/opt/trn_rl_repo", "/root/.axon_site/_ro/trn_rl_repo"):
    if os.path.isdir(p) and p not in sys.path:
        sys.path.insert(0, p)

import concourse.bass as bass  # noqa: E402,F401
import concourse.bacc as bacc  # noqa: E402
import concourse.mybir as mybir  # noqa: E402
import concourse.tile as tile  # noqa: E402
from concourse import bass_utils  # noqa: E402

F32 = mybir.dt.float32
F16 = mybir.dt.float16
I16 = mybir.dt.int16

# ---- geometry constants (mirror of the reference) ----
VOL = 256
N_ANGLES = 180
N_DET = 363
N_SAMPLES = 363
CEN = (VOL - 1) / 2.0
DCEN = (N_DET - 1) / 2.0
SCEN = (N_SAMPLES - 1) / 2.0

N_U = 16          # 16-row windows (vertical pair of 8-row phases)
XW = 7            # x-slots covered per cell (entry spans XW+1 pixels)
N_XQ = 37         # ceil(257/7)
HALF = N_U * N_XQ             # 592 entries per orientation
NUM_ELEMS = 2 * HALF          # 1184: [0,592) normal, [592,1184) transposed
D_GATHER = 8                  # f32 lanes = 16 fp16: rows 16u+w and 16u+8+w

N_CORES = 8
TARGET_COLS = 448    # cells per chunk (approx)
IB = 8               # chunks per idx-batch DMA

_plan_cache = {}
_compile_cache = {}


def _f32(v):
    return np.float32(v)


def _ray_cells(k):
    """Merged wide-cell decomposition for angle index k.

    Returns a list over detectors d of (cellidx[int], W[8, 16, ncell]); slot
    8*h + j (h = half) weights pixel img[16u + 8*h + (w), 7*xq+j].
    """
    theta = _f32(k) * _f32(np.pi / N_ANGLES)
    c = np.cos(theta, dtype=np.float32)
    s = np.sin(theta, dtype=np.float32)
    flip = abs(s) > abs(c)
    ce, se = (s, c) if flip else (c, s)
    d = np.arange(N_DET, dtype=np.float32)[:, None] - _f32(DCEN)
    t = np.arange(N_SAMPLES, dtype=np.float32)[None, :] - _f32(SCEN)
    fx = (ce * d - se * t + _f32(CEN)).astype(np.float32)
    fy = (se * d + ce * t + _f32(CEN)).astype(np.float32)
    x0 = np.floor(fx).astype(np.int64)
    y0 = np.floor(fy).astype(np.int64)
    wx = (fx - x0).astype(np.float64)
    wy = (fy - y0).astype(np.float64)

    x_dead = (x0 < -1) | (x0 > 255)
    xslot = np.clip(x0, 0, 256)
    a0 = np.where((x0 >= 0) & (x0 < VOL), 1.0 - wx, 0.0)
    a1 = np.where((x0 + 1 >= 0) & (x0 + 1 < VOL), wx, 0.0)
    neg1 = x0 == -1
    a0 = np.where(neg1, wx, a0)
    a1 = np.where(neg1, 0.0, a1)

    lo = (~x_dead) & (y0 >= 0) & (y0 < VOL)
    hi = (~x_dead) & (y0 + 1 >= 0) & (y0 + 1 < VOL)

    D_idx = np.broadcast_to(np.arange(N_DET)[:, None], fx.shape)
    d_all = np.concatenate([D_idx[lo], D_idx[hi]])
    y_all = np.concatenate([y0[lo], y0[hi] + 1])
    tw_all = np.concatenate([(1.0 - wy)[lo], wy[hi]])
    a0_all = np.concatenate([a0[lo], a0[hi]])
    a1_all = np.concatenate([a1[lo], a1[hi]])
    xs_all = np.concatenate([xslot[lo], xslot[hi]])

    base = HALF if flip else 0
    u = y_all >> 4
    w = y_all & 7
    h = (y_all >> 3) & 1
    xq = xs_all // XW
    j0 = xs_all - xq * XW
    gkey = (d_all * N_U + u) * N_XQ + xq
    cells, inv = np.unique(gkey, return_inverse=True)
    ncell = len(cells)
    W = np.zeros((8, 16, ncell))
    np.add.at(W, (w, 8 * h + j0, inv), tw_all * a0_all)
    np.add.at(W, (w, 8 * h + j0 + 1, inv), tw_all * a1_all)
    cell_d = cells // (N_U * N_XQ)
    cell_e = cells % (N_U * N_XQ) + base
    bounds = np.searchsorted(cell_d, np.arange(N_DET + 1))
    out = []
    for dd in range(N_DET):
        i0, i1 = bounds[dd], bounds[dd + 1]
        out.append((cell_e[i0:i1], W[:, :, i0:i1]))
    return out


def _build_plan():
    """Geometry-only precompute shared across calls."""
    if "plan" in _plan_cache:
        return _plan_cache["plan"]
    rays = {}   # k -> per-detector (cells, W)
    cnts = np.zeros((N_ANGLES, N_DET), dtype=np.int64)
    for k in range(N_ANGLES):
        rc = _ray_cells(k)
        rays[k] = rc
        cnts[k] = [len(r[0]) for r in rc]

    core_ids = [list(range(c, N_ANGLES, N_CORES)) for c in range(N_CORES)]

    stream_rays = {}
    NR0 = 0
    for c in range(N_CORES):
        ids = core_ids[c]
        A, D = np.meshgrid(ids, np.arange(N_DET), indexing="ij")
        A, D = A.ravel(), D.ravel()
        lens = cnts[A, D]
        o = np.argsort(-lens, kind="stable")
        A, D, lens = A[o], D[o], lens[o]
        # every group's table is identical, so deal rays round-robin by length
        # to equalize the per-stream sorted profiles (minimal chunk padding)
        for g in range(8):
            stream_rays[(c, g)] = (A[g::8], D[g::8], lens[g::8])
            NR0 = max(NR0, len(A[g::8]))

    # global sorted length profile (max across all streams at each position)
    P = np.zeros(NR0, dtype=np.int64)
    for (c, g), (A, D, lens) in stream_rays.items():
        P[:len(lens)] = np.maximum(P[:len(lens)], lens)

    # chunk schedule: variable CH, L multiple of 16, ~TARGET_COLS per chunk
    chunks = []   # (p0, CHk, Lk, coloff)
    coloff = 0
    p = 0
    from math import gcd
    nz = int(np.count_nonzero(P))
    while p < nz:
        # CHk a multiple of 8 and Lk even: NCH = CHk*Lk is a multiple of 32
        # (idx stream 4B alignment) and stays SBUF-sized.
        Lk = (int(P[p]) + 1) // 2 * 2
        CHk = max(8, (TARGET_COLS // max(Lk, 1)) // 8 * 8)
        CHk = min(CHk, nz - p)
        Lk = (int(P[p:p + CHk].max()) + 1) // 2 * 2
        need = 32 // gcd(CHk, 32)
        Lk = (Lk + need - 1) // need * need
        chunks.append((p, CHk, Lk, coloff))
        coloff += CHk * Lk
        p += CHk
    NRall = NR0
    Ntot = coloff

    colstart = np.zeros(NRall, dtype=np.int64)
    for (p0, CHk, Lk, off) in chunks:
        for r in range(CHk):
            colstart[p0 + r] = off + r * Lk

    cores = []
    for c in range(N_CORES):
        SL = 2 * D_GATHER
        idxs = np.zeros((8, Ntot), dtype=np.int16)
        W8 = np.zeros((8, 8, Ntot, SL), dtype=np.float32)  # [g, wrow, cell, slot]
        for g in range(8):
            A, D, lens = stream_rays[(c, g)]
            for pos in range(len(A)):
                a, dd, n = A[pos], D[pos], lens[pos]
                if n == 0:
                    continue
                cells, W = rays[a][dd]
                col = colstart[pos]
                idxs[g, col:col + n] = cells.astype(np.int16)
                W8[g, :, col:col + n, :] = W.transpose(0, 2, 1)

        # replicate weights across the 2 batches: partition 16g+8b+w
        W128 = np.zeros((128, Ntot, SL), dtype=np.float16)
        for g in range(8):
            for b in range(2):
                W128[16 * g + 8 * b:16 * g + 8 * b + 8] = W8[g]
        idxw = np.zeros((128, Ntot // 16), dtype=np.int16)
        for g in range(8):
            idxw[16 * g:16 * g + 16, :] = idxs[g].reshape(Ntot // 16, 16).T

        # repack chunk-major contiguous: w01flat (fp16), idxflat (int16)
        w01flat = np.empty(128 * SL * Ntot, dtype=np.float16)
        for (p0, CHk, Lk, off) in chunks:
            NCH = CHk * Lk
            blk = W128[:, off:off + NCH, :].reshape(128, SL * NCH)
            w01flat[SL * 128 * off:SL * 128 * (off + NCH)] = blk.reshape(-1)
        idxflat = np.empty(128 * (Ntot // 16), dtype=np.int16)
        ioff = 0
        kk = 0
        while kk < len(chunks):
            ke = min(kk + IB, len(chunks))
            o0 = chunks[kk][3]
            o1 = chunks[ke - 1][3] + chunks[ke - 1][1] * chunks[ke - 1][2]
            iblk = idxw[:, o0 // 16:o1 // 16]
            n16 = iblk.shape[1]
            idxflat[ioff:ioff + 128 * n16] = iblk.reshape(-1)
            ioff += 128 * n16
            kk = ke
        raymap = []
        for g in range(8):
            A, D, lens = stream_rays[(c, g)]
            raymap.append((A, D))
        cores.append(dict(idxflat=idxflat, w01flat=w01flat, raymap=raymap))

    ones = np.zeros((128, 16), dtype=np.float32)
    for g in range(8):
        for b in range(2):
            ones[16 * g + 8 * b:16 * g + 8 * b + 8, 2 * g + b] = 1.0

    plan = dict(chunks=chunks, NR=NRall, Ntot=Ntot, cores=cores, ones=ones)
    _plan_cache["plan"] = plan
    return plan


def _build_tables(x):
    """x [2,256,256] -> tbl [128, NUM_ELEMS*D_GATHER] f32 (8xfp16 per entry).

    Entry e in [0,1221): normal, e = u*37 + xq: partition 16g+8b+w holds
    fp16 pixels img[b, 8u+w, 7*xq + j] for j = 0..7 (zero-padded outside).
    Entry 1221+e: same with the transposed image.
    """
    SL = 2 * D_GATHER
    tbl16 = np.zeros((16, NUM_ELEMS, SL), dtype=np.float16)
    for ori, im4 in enumerate((x, x.transpose(0, 2, 1))):
        img = np.zeros((2, 16 * N_U, XW * N_XQ + 1), dtype=np.float32)
        img[:, :VOL, :VOL] = im4
        for b in range(2):
            for w in range(8):
                lo = img[b, w::16, :]                      # rows 16u+w   [16, 260]
                hi = img[b, 8 + w::16, :]                  # rows 16u+8+w [16, 260]
                ent = np.concatenate(
                    [np.stack([lo[:, 7 * q:7 * q + 8] for q in range(N_XQ)], 1),
                     np.stack([hi[:, 7 * q:7 * q + 8] for q in range(N_XQ)], 1)],
                    axis=-1)                               # [16, 37, 16]
                tbl16[8 * b + w, ori * HALF:(ori + 1) * HALF] = (
                    ent.reshape(HALF, SL).astype(np.float16))
    return np.tile(tbl16, (8, 1, 1)).reshape(128, NUM_ELEMS * SL).view(np.float32)


def _radon_kernel(tc, outs, ins, *, chunks, NR):
    ctx = ExitStack()
    with ctx:
        nc = tc.nc
        tbl_d, idx_d, w01_d, ones_d = ins
        out16_d = outs[0]

        const_pool = ctx.enter_context(tc.tile_pool(name="const", bufs=1))
        g_pool = ctx.enter_context(tc.tile_pool(name="g", bufs=4))
        w_pool = ctx.enter_context(tc.tile_pool(name="w", bufs=3))
        t_pool = ctx.enter_context(tc.tile_pool(name="t", bufs=1))
        i_pool = ctx.enter_context(tc.tile_pool(name="i", bufs=2))
        acc_pool = ctx.enter_context(tc.tile_pool(name="acc", bufs=1))
        psum_pool = ctx.enter_context(tc.tile_pool(name="ps", bufs=2, space="PSUM"))

        tbl = const_pool.tile([128, NUM_ELEMS * D_GATHER], F32)
        nc.sync.dma_start(tbl[:], tbl_d[:])
        ones = const_pool.tile([128, 16], F32)
        nc.sync.dma_start(ones[:], ones_d[:])

        acc = acc_pool.tile([128, NR], F32)
        nc.vector.memset(acc[:], 0.0)

        # idx batches: IB chunks per DMA
        nchunks = len(chunks)
        batches = []
        k = 0
        ioff = 0
        while k < nchunks:
            ke = min(k + IB, nchunks)
            n16 = sum(chunks[j][1] * chunks[j][2] // 16 for j in range(k, ke))
            batches.append((k, ke, ioff, n16))
            ioff += 128 * n16
            k = ke

        outs_sb = acc_pool.tile([16, NR], F32)
        nc.vector.memset(outs_sb[:], 0.0)

        for (k0, k1, ioff, n16) in batches:
            ib = i_pool.tile([128, n16], I16, tag="idx")
            nc.sync.dma_start(
                ib[:], idx_d[ioff:ioff + 128 * n16].rearrange("(p n) -> p n", p=128))
            sl = 0
            for j in range(k0, k1):
                p0, CHk, Lk, off = chunks[j]
                NCH = CHk * Lk
                SL = 2 * D_GATHER
                w01 = w_pool.tile([128, SL * NCH], F16, tag="w01")
                nc.sync.dma_start(
                    w01[:],
                    w01_d[SL * 128 * off:SL * 128 * (off + NCH)]
                    .rearrange("(p n) -> p n", p=128))

                gt = g_pool.tile([128, D_GATHER * NCH], F32, tag="g")
                nc.gpsimd.ap_gather(
                    gt[:], tbl[:], ib[:, sl:sl + NCH // 16],
                    channels=128, num_elems=NUM_ELEMS, d=D_GATHER, num_idxs=NCH,
                )
                gh = gt[:].bitcast(F16)    # [128, SL*NCH] fp16, cell-major
                # t01 fp16 layout: [0,16N) products, [16N,24N) tree lvl1,
                # [24N,28N) lvl2, [28N,30N) lvl3.
                t01 = t_pool.tile([128, 30 * NCH], F16, tag="t01")
                prod = t01[:, 0:16 * NCH].rearrange("p (c s) -> p c s", s=16)
                nc.vector.tensor_mul(t01[:, 0:16 * NCH], w01[:], gh)
                # slot add-tree at fp16 2x rate: 16 -> 8 -> 4 -> 2
                l1 = t01[:, 16 * NCH:24 * NCH].rearrange("p (c s) -> p c s", s=8)
                nc.vector.tensor_add(l1, prod[:, :, 0:8], prod[:, :, 8:16])
                l2 = t01[:, 24 * NCH:28 * NCH].rearrange("p (c s) -> p c s", s=4)
                nc.vector.tensor_add(l2, l1[:, :, 0:4], l1[:, :, 4:8])
                l3 = t01[:, 28 * NCH:30 * NCH].rearrange("p (c s) -> p c s", s=2)
                nc.vector.tensor_add(l3, l2[:, :, 0:2], l2[:, :, 2:4])
                # per-ray fold: reduce the 2*Lk remaining values per ray slot
                nc.vector.tensor_reduce(
                    acc[:, p0:p0 + CHk],
                    t01[:, 28 * NCH:30 * NCH]
                    .rearrange("p (r tl) -> p r tl", tl=2 * Lk),
                    axis=mybir.AxisListType.X,
                    op=mybir.AluOpType.add,
                )
                sl += NCH // 16

            # fold this batch's finished ray positions on the idle TensorE
            b_p0 = chunks[k0][0]
            b_p1 = chunks[k1 - 1][0] + chunks[k1 - 1][1]
            NMM = 512
            for m0 in range(b_p0, b_p1, NMM):
                m1 = min(m0 + NMM, b_p1)
                ps = psum_pool.tile([16, m1 - m0], F32, tag="ps")
                nc.tensor.matmul(ps[:], ones[:], acc[:, m0:m1], start=True, stop=True)
                nc.scalar.copy(outs_sb[:, m0:m1], ps[:])
        nc.sync.dma_start(out16_d[:], outs_sb[:])


def _compile(plan):
    key = "nc"
    if key in _compile_cache:
        return _compile_cache[key]
    Ntot, NR = plan["Ntot"], plan["NR"]
    nc = bacc.Bacc("TRN2", target_bir_lowering=False, debug=False,
                   enable_asserts=False, num_devices=N_CORES)
    tbl_d = nc.dram_tensor("tbl", [128, NUM_ELEMS * D_GATHER], F32,
                           kind="ExternalInput").ap()
    idx_d = nc.dram_tensor("idxf", [128 * (Ntot // 16)], I16, kind="ExternalInput").ap()
    w01_d = nc.dram_tensor("w01f", [128 * 2 * D_GATHER * Ntot], F16,
                           kind="ExternalInput").ap()
    ones_d = nc.dram_tensor("ones", [128, 16], F32, kind="ExternalInput").ap()
    out16_d = nc.dram_tensor("out16", [16, NR], F32, kind="ExternalOutput").ap()
    with tile.TileContext(nc) as tc:
        _radon_kernel(tc, [out16_d], [tbl_d, idx_d, w01_d, ones_d],
                      chunks=plan["chunks"], NR=plan["NR"])
    nc.compile()
    _compile_cache[key] = nc
    return nc


def kernel(x):
    """x [2,256,256,1] f32 -> sinogram [2,180,363,1] f32."""
    x = np.asarray(x, dtype=np.float32)
    plan = _build_plan()
    tbl = _build_tables(x[:, :, :, 0])
    nc = _compile(plan)
    in_maps = []
    for c in range(N_CORES):
        st = plan["cores"][c]
        in_maps.append(dict(tbl=tbl, idxf=st["idxflat"], w01f=st["w01flat"],
                            ones=plan["ones"]))
    res = bass_utils.run_bass_kernel_spmd(nc, in_maps, core_ids=list(range(N_CORES)))
    sino = np.zeros((2, N_ANGLES, N_DET), dtype=np.float32)
    for c in range(N_CORES):
        out16 = res.results[c]["out16"]
        for g in range(8):
            A, D = plan["cores"][c]["raymap"][g]
            n = len(A)
            for b in range(2):
                sino[b, A, D] = out16[2 * g + b, :n]
    return sino[..., None]


if __name__ == "__main__":
    import time
    x = np.load("/tmp/x.npy")
    t0 = time.time()
    out = kernel(x)
    print("kernel() wall time:", time.time() - t0)
    exp = np.load("/tmp/expected_np.npy")
    rel = np.linalg.norm((out - exp).ravel()) / np.linalg.norm(exp.ravel())
    print("rel l2 vs numpy ref:", rel)



# revision 11
# speedup vs baseline: 1.6649x; 1.6649x over previous
"""Trainium2 Bass kernel v9: 2D parallel-beam forward projection (Radon).

Input:  x [2, 256, 256, 1] float32
Output: sinogram [2, 180, 363, 1] float32

Strategy (8 NeuronCores, SPMD), v9 = DMA-descriptor gather:
  - Angles interleaved across cores (core c: angles c::8). Per core, the
    ~8167 rays are dealt (longest-first round-robin) onto 128 partition
    lanes; each lane owns whole rays.
  - Cell = (16-row window u, 7-px x-window xq) of the (possibly
    transposed) image, as in v7: per-angle transpose trick keeps the
    ray's y-step >= 0.707, so a ray has ~20 cells.
  - Pixels: one HBM table gtbl[1184, 256] f16; row (ori, u, xq) holds
    the cell's 2 batches x 8 w-rows x (2h x 8j) pixels. nc.gpsimd.
    dma_gather streams cells: one 512B DMA descriptor per cell, index
    i -> partition i%128, so each lane receives its own cells in order.
    Descriptor generation is ~0.34ns/cell on GPSIMD and the copies run
    on the 16 DMA engines - the Q7 ap_gather bottleneck (34ns/idx) of
    v7/v8 is gone.
  - Weights (geometry-only, cached across calls): 128 f16 per cell
    (w x 2h x 8j), applied to both batches by two DVE muls.
  - DVE per chunk: 2 muls (fp16 2x), 7-level pairwise add tree (fp16
    2x) folding 128 -> 1 per (cell, batch), then one segmented
    tensor_reduce folding each ray's Lk cells into acc[128, 2*NR].
"""
import os
import sys
from contextlib import ExitStack

import numpy as np

for p in ("/opt/trn_rl_repo", "/root/.axon_site/_ro/trn_rl_repo"):
    if os.path.isdir(p) and p not in sys.path:
        sys.path.insert(0, p)

import concourse.bass as bass  # noqa: E402,F401
import concourse.bacc as bacc  # noqa: E402
import concourse.mybir as mybir  # noqa: E402
import concourse.tile as tile  # noqa: E402
from concourse import bass_utils  # noqa: E402

F32 = mybir.dt.float32
F16 = mybir.dt.float16
I16 = mybir.dt.int16

# ---- geometry constants (mirror of the reference) ----
VOL = 256
N_ANGLES = 180
N_DET = 363
N_SAMPLES = 363
CEN = (VOL - 1) / 2.0
DCEN = (N_DET - 1) / 2.0
SCEN = (N_SAMPLES - 1) / 2.0

N_U = 16          # 16-row windows
XW = 7            # x-slots covered per cell (entry spans XW+1 pixels)
N_XQ = 37         # ceil(257/7)
HALF = N_U * N_XQ             # 592 entries per orientation
NUM_ELEMS = 2 * HALF          # 1184 table rows
CSL = 128                     # weight slots per cell (8w x 2h x 8j)
ESL = 256                     # table row length in f16 (2 batches x CSL)

N_CORES = 8
N_LANES = 128
TARGET_C = 64    # cell-columns per chunk (<= MAX_C)
MAX_C = 64

_plan_cache = {}
_compile_cache = {}


def _f32(v):
    return np.float32(v)


def _ray_cells(k):
    """Merged cell decomposition for angle index k.

    Returns a list over detectors d of (cellidx[int], W[8, 16, ncell]); W
    slot 8*h + j (h = y-half) weights pixel img[16u + 8h + (w), 7*xq+j].
    """
    theta = _f32(k) * _f32(np.pi / N_ANGLES)
    c = np.cos(theta, dtype=np.float32)
    s = np.sin(theta, dtype=np.float32)
    flip = abs(s) > abs(c)
    ce, se = (s, c) if flip else (c, s)
    d = np.arange(N_DET, dtype=np.float32)[:, None] - _f32(DCEN)
    t = np.arange(N_SAMPLES, dtype=np.float32)[None, :] - _f32(SCEN)
    fx = (ce * d - se * t + _f32(CEN)).astype(np.float32)
    fy = (se * d + ce * t + _f32(CEN)).astype(np.float32)
    x0 = np.floor(fx).astype(np.int64)
    y0 = np.floor(fy).astype(np.int64)
    wx = (fx - x0).astype(np.float64)
    wy = (fy - y0).astype(np.float64)

    x_dead = (x0 < -1) | (x0 > 255)
    xslot = np.clip(x0, 0, 256)
    a0 = np.where((x0 >= 0) & (x0 < VOL), 1.0 - wx, 0.0)
    a1 = np.where((x0 + 1 >= 0) & (x0 + 1 < VOL), wx, 0.0)
    neg1 = x0 == -1
    a0 = np.where(neg1, wx, a0)
    a1 = np.where(neg1, 0.0, a1)

    lo = (~x_dead) & (y0 >= 0) & (y0 < VOL)
    hi = (~x_dead) & (y0 + 1 >= 0) & (y0 + 1 < VOL)

    D_idx = np.broadcast_to(np.arange(N_DET)[:, None], fx.shape)
    d_all = np.concatenate([D_idx[lo], D_idx[hi]])
    y_all = np.concatenate([y0[lo], y0[hi] + 1])
    tw_all = np.concatenate([(1.0 - wy)[lo], wy[hi]])
    a0_all = np.concatenate([a0[lo], a0[hi]])
    a1_all = np.concatenate([a1[lo], a1[hi]])
    xs_all = np.concatenate([xslot[lo], xslot[hi]])

    base = HALF if flip else 0
    u = y_all >> 4
    w = y_all & 7
    h = (y_all >> 3) & 1
    xq = xs_all // XW
    j0 = xs_all - xq * XW
    gkey = (d_all * N_U + u) * N_XQ + xq
    cells, inv = np.unique(gkey, return_inverse=True)
    ncell = len(cells)
    W = np.zeros((8, 16, ncell))
    np.add.at(W, (w, 8 * h + j0, inv), tw_all * a0_all)
    np.add.at(W, (w, 8 * h + j0 + 1, inv), tw_all * a1_all)
    cell_d = cells // (N_U * N_XQ)
    cell_e = cells % (N_U * N_XQ) + base
    bounds = np.searchsorted(cell_d, np.arange(N_DET + 1))
    out = []
    for dd in range(N_DET):
        i0, i1 = bounds[dd], bounds[dd + 1]
        out.append((cell_e[i0:i1], W[:, :, i0:i1]))
    return out


def _build_plan():
    """Geometry-only precompute shared across calls (input-independent)."""
    if "plan" in _plan_cache:
        return _plan_cache["plan"]
    rays = {}
    cnts = np.zeros((N_ANGLES, N_DET), dtype=np.int64)
    for k in range(N_ANGLES):
        rc = _ray_cells(k)
        rays[k] = rc
        cnts[k] = [len(r[0]) for r in rc]

    cores = []
    NRmax = 0
    lane_rays_all = []
    for c in range(N_CORES):
        ids = list(range(c, N_ANGLES, N_CORES))
        A, D = np.meshgrid(ids, np.arange(N_DET), indexing="ij")
        A, D = A.ravel(), D.ravel()
        lens = cnts[A, D]
        o = np.argsort(-lens, kind="stable")
        A, D, lens = A[o], D[o], lens[o]
        lane_rays = [(A[p::N_LANES], D[p::N_LANES], lens[p::N_LANES])
                     for p in range(N_LANES)]
        lane_rays_all.append(lane_rays)
        NRmax = max(NRmax, max(len(a) for a, _, _ in lane_rays))

    # global sorted length profile (max across all lanes of all cores)
    P = np.zeros(NRmax, dtype=np.int64)
    for lane_rays in lane_rays_all:
        for A, D, lens in lane_rays:
            P[:len(lens)] = np.maximum(P[:len(lens)], lens)
    nz = int(np.count_nonzero(P))
    NR = nz

    # chunk schedule over ray positions: chunk = CHk rays x Lk cells
    chunks = []   # (p0, CHk, Lk, coloff)
    coloff = 0
    p = 0
    while p < nz:
        Lk = int(P[p])
        CHk = max(1, min(TARGET_C // max(Lk, 1), nz - p))
        CHk = min(CHk, MAX_C // max(Lk, 1)) or 1
        Lk = int(P[p:p + CHk].max())
        while CHk > 1 and CHk * Lk > MAX_C:
            CHk -= 1
            Lk = int(P[p:p + CHk].max())
        chunks.append((p, CHk, Lk, coloff))
        coloff += CHk * Lk
        p += CHk
    Ntot = coloff

    colstart = np.zeros(nz, dtype=np.int64)
    for (p0, CHk, Lk, off) in chunks:
        for r in range(CHk):
            colstart[p0 + r] = off + r * Lk

    for c in range(N_CORES):
        lane_rays = lane_rays_all[c]
        idxs = np.zeros((N_LANES, Ntot), dtype=np.int16)
        wts = np.zeros((N_LANES, Ntot, CSL), dtype=np.float16)
        raymap = []
        for pl in range(N_LANES):
            A, D, lens = lane_rays[pl]
            for pos in range(len(A)):
                a, dd, n = A[pos], D[pos], lens[pos]
                if n == 0:
                    continue
                cells, W = rays[a][dd]
                col = colstart[pos]
                idxs[pl, col:col + n] = cells.astype(np.int16)
                # W [8w, 16slot, ncell] -> [ncell, 128]
                wts[pl, col:col + n, :] = (
                    W.reshape(CSL, n).T.astype(np.float16))
            raymap.append((A, D))

        # idx stream for dma_gather: index i = c*128 + p -> lane p col c.
        # SBUF layout [128, Ntot*128/16]: flat j -> (j%16, j//16), replicated
        # to all 8 16-partition groups.
        idxflat = idxs.T.reshape(-1)  # [Ntot*128]: col-major (c, p)
        idxw = np.zeros((128, Ntot * 128 // 16), dtype=np.int16)
        blk = idxflat.reshape(-1, 16).T   # [16, Ntot*8]
        for g in range(8):
            idxw[16 * g:16 * g + 16] = blk
        # weight stream: [128 lanes, Ntot*CSL] f16, chunk-contiguous already
        # (cells are column-positions; DMA slices [*, off*CSL:(off+C)*CSL]).
        w01flat = wts.reshape(N_LANES, Ntot * CSL)
        cores.append(dict(idxw=idxw, w01flat=w01flat, raymap=raymap))

    plan = dict(chunks=chunks, NR=NR, Ntot=Ntot, cores=cores)
    _plan_cache["plan"] = plan
    return plan


def _build_gtbl(x):
    """x [2,256,256] -> gtbl [NUM_ELEMS, 256] f16 HBM gather table.

    Row (ori*592 + u*37 + xq), col b*128 + w*16 + h*8 + j =
    img_ori[b, 16u + 8h + w, 7*xq + j] (zero-padded outside).
    """
    out = np.zeros((2, N_U, N_XQ, 2, 8, 2, 8), dtype=np.float16)
    for ori, im in enumerate((x, x.transpose(0, 2, 1))):
        img = np.zeros((2, 16 * N_U, XW * N_XQ + 1), dtype=np.float32)
        img[:, :VOL, :VOL] = im
        # [b, u, h, w, xq, j]
        v = img[:, :, :XW * N_XQ].reshape(2, N_U, 2, 8, N_XQ, XW)
        # j spans 8 px: 7*xq .. 7*xq+7 -> last col of next window start
        nxt = img[:, :, 7::7].reshape(2, N_U, 2, 8, N_XQ)
        out[ori, :, :, :, :, :, :XW] = v.transpose(1, 4, 0, 3, 2, 5)
        out[ori, :, :, :, :, :, XW] = nxt.transpose(1, 4, 0, 3, 2)
    return out.reshape(NUM_ELEMS, ESL)


def _radon_kernel(tc, outs, ins, *, chunks, NR, Ntot):
    ctx = ExitStack()
    with ctx:
        nc = tc.nc
        gtbl_d, idx_d, w01_d = ins
        acc_d = outs[0]

        const_pool = ctx.enter_context(tc.tile_pool(name="const", bufs=1))
        g_pool = ctx.enter_context(tc.tile_pool(name="g", bufs=2))
        w_pool = ctx.enter_context(tc.tile_pool(name="w", bufs=2))
        t_pool = ctx.enter_context(tc.tile_pool(name="t", bufs=1))
        acc_pool = ctx.enter_context(tc.tile_pool(name="acc", bufs=1))

        idx_sb = const_pool.tile([128, Ntot * 8], I16)
        nc.sync.dma_start(idx_sb[:], idx_d[:])

        acc = acc_pool.tile([128, 2 * NR], F32)
        nc.vector.memset(acc[:], 0.0)

        qn = 0
        for (p0, CHk, Lk, off) in chunks:
            C = CHk * Lk
            gt = g_pool.tile([128, ESL * C], F16, tag="g")
            # SWDGE ring caps a single gather at ~1024 descriptors; split
            # into <=8-column sub-gathers round-robined over the 4 queues.
            for c0 in range(0, C, 8):
                cw = min(8, C - c0)
                nc.gpsimd.dma_gather(
                    gt[:, ESL * c0:ESL * (c0 + cw)]
                    .rearrange("p (c e) -> p c e", e=ESL),
                    gtbl_d[:],
                    idx_sb[:, (off + c0) * 8:(off + c0 + cw) * 8],
                    num_idxs=128 * cw,
                    num_idxs_reg=128 * cw,
                    elem_size=ESL,
                    queue_num=qn,
                )
                qn = (qn + 1) % 4
            wt = w_pool.tile([128, CSL * C], F16, tag="w")
            nc.sync.dma_start(
                wt[:], w01_d[:, off * CSL:(off + C) * CSL])

            # prod [0, 256C); tree levels packed at [256C, 510C)
            st = t_pool.tile([128, 510 * C], F16, tag="t")
            g3 = gt[:].rearrange("p (c e) -> p c e", e=ESL)
            w3 = wt[:].rearrange("p (c e) -> p c e", e=CSL)
            pr = st[:, 0:ESL * C].rearrange("p (c e) -> p c e", e=ESL)
            nc.vector.tensor_mul(pr[:, :, 0:CSL], g3[:, :, 0:CSL], w3)
            nc.vector.tensor_mul(pr[:, :, CSL:ESL], g3[:, :, CSL:ESL], w3)
            # pairwise add tree over the 128 slots of each (cell, batch)
            # half-cells: 2C blocks of 128 -> 64 -> ... -> 1
            src = st[:, 0:ESL * C].rearrange("p (c e) -> p c e", e=CSL)
            base = ESL * C
            n = CSL // 2
            while n >= 1:
                dst = st[:, base:base + 2 * C * n].rearrange(
                    "p (c e) -> p c e", e=n)
                nc.vector.tensor_add(dst, src[:, :, 0:n], src[:, :, n:2 * n])
                src = dst
                base += 2 * C * n
                n //= 2
            # src is [p, 2C, 1]: per (cell, batch) sums at stride 2 per cell.
            # fold each ray's Lk cells: in [p, CHk, b(2), Lk], out [p, CHk, 2]
            nc.vector.tensor_reduce(
                acc[:, 2 * p0:2 * (p0 + CHk)]
                .rearrange("p (r b) -> p r b", b=2),
                st[:, base - 2 * C:base]
                .rearrange("p (r l b) -> p r b l", b=2, l=Lk),
                axis=mybir.AxisListType.X,
                op=mybir.AluOpType.add,
            )
        nc.sync.dma_start(acc_d[:], acc[:])


def _compile(plan):
    key = "nc9"
    if key in _compile_cache:
        return _compile_cache[key]
    Ntot, NR = plan["Ntot"], plan["NR"]
    nc = bacc.Bacc("TRN2", target_bir_lowering=False, debug=False,
                   enable_asserts=False, num_devices=N_CORES,
                   num_swdge_queues=4)
    gtbl_d = nc.dram_tensor("gtbl", [NUM_ELEMS, ESL], F16,
                            kind="ExternalInput").ap()
    idx_d = nc.dram_tensor("idxw", [128, Ntot * 8], I16,
                           kind="ExternalInput").ap()
    w01_d = nc.dram_tensor("w01f", [128, Ntot * CSL], F16,
                           kind="ExternalInput").ap()
    acc_d = nc.dram_tensor("acc", [128, 2 * NR], F32,
                           kind="ExternalOutput").ap()
    with tile.TileContext(nc) as tc:
        _radon_kernel(tc, [acc_d], [gtbl_d, idx_d, w01_d],
                      chunks=plan["chunks"], NR=plan["NR"], Ntot=plan["Ntot"])
    nc.compile()
    _compile_cache[key] = nc
    return nc


def kernel(x):
    """x [2,256,256,1] f32 -> sinogram [2,180,363,1] f32."""
    x = np.asarray(x, dtype=np.float32)
    plan = _build_plan()
    gtbl = _build_gtbl(x[:, :, :, 0])
    nc = _compile(plan)
    in_maps = []
    for c in range(N_CORES):
        st = plan["cores"][c]
        in_maps.append(dict(gtbl=gtbl, idxw=st["idxw"], w01f=st["w01flat"]))
    res = bass_utils.run_bass_kernel_spmd(nc, in_maps, core_ids=list(range(N_CORES)))
    NR = plan["NR"]
    sino = np.zeros((2, N_ANGLES, N_DET), dtype=np.float32)
    for c in range(N_CORES):
        accv = res.results[c]["acc"]
        for pl in range(N_LANES):
            A, D = plan["cores"][c]["raymap"][pl]
            n = min(len(A), NR)   # positions >= NR are zero-cell rays
            for b in range(2):
                sino[b, A[:n], D[:n]] = accv[pl, 2 * np.arange(n) + b]
    return sino[..., None]


if __name__ == "__main__":
    import time
    x = np.load("/tmp/x.npy")
    t0 = time.time()
    out = kernel(x)
    print("kernel() wall time:", time.time() - t0)
    exp = np.load("/tmp/expected_np.npy")
    rel = np.linalg.norm((out - exp).ravel()) / np.linalg.norm(exp.ravel())
    print("rel l2 vs numpy ref:", rel)
